# revision 29
# baseline (speedup 1.0000x reference)
"""Trainium2 Bass kernel for nn_AudioSegmentHandler (scatter_memory).

Semantics (matches the reference):
  1. Linear-interpolate each row's generated_audio [24000] down to
     gap_length=16000 (torch F.interpolate align_corners=False). Since
     24000/16000 == 1.5 exactly, the gather pattern is a fixed stride-3
     / stride-2 stencil:
        out[2k]   = 0.75*g[3k]   + 0.25*g[3k+1]
        out[2k+1] = 0.25*g[3k+1] + 0.75*g[3k+2]
  2. Crossfade: first 1000 samples *= linspace(0,1,1000), last 1000
     *= linspace(1,0,1000).
  3. For each row, sequentially scatter-write the 16000-sample segment
     into the audio at the 8 (sorted) gap_starts offsets; later gaps
     overwrite earlier ones on overlap.

Distribution: pure data-parallel, batch 32 -> 8 NeuronCores x 4 rows.

Perf strategy (vs the f32 baseline at ~145us):
  - fp16 transport end-to-end (gate is rel_err < 2e-2; fp16 adds
    ~1e-3): halves the dominant DRAM->DRAM copy to 15.4MB per core.
  - Copy issued first (sync queue), chunked and throttled to <=4
    chunks in flight so other queues' small DMAs interleave within a
    couple of 60KB descriptors instead of a deep descriptor backlog.
  - The crossfade tile is computed arithmetically on all 128
    partitions (position q = iota mod 16000) -- no DMA replication on
    the critical path.
  - Scatter rows 0,1 (early copies): ordered chains (overlap
    semantics), one per queue (sync-after-copy / scalar); latency
    hides under the copy of later rows.
  - Scatter rows 2,3 (last copies): ORDER-FREE. In DRAM scratch we
    precompute per-gap contents c_g such that overlapping gap writes
    carry identical bytes: backward over the sorted gaps,
        c_7 = seg;  c_g = seg overwritten on [d_g, d_g+G) by c_{g+1},
    d_g = min(gap[g+1]-gap[g], G) (dynamic DMA offset; writes beyond
    G land in a pad half). All 8 writes of the row then commute, so
    after the row's copy lands they issue back-to-back with no
    inter-write semaphores, split across two queues: the post-copy
    tail is ~5us instead of ~8x3us.
  - Engine instruction issue is ~0.6-1us each, so per-queue
    instruction counts are minimized: segments staged once per row to
    DRAM (single-descriptor 32KB sources), c_g inits are one
    broadcast-AP DMA per row, final writes hoist their offset-register
    loads and use one combined semaphore wait.
"""

import numpy as np

B = 32
T = 1920000
L = 24000  # generated_audio length
G = 16000  # gap length
N_GAPS = 8
N_CORES = 8
R = B // N_CORES  # rows per core
N_CHUNK = 8  # copy chunks per row
MAX_INFLIGHT = 4  # copy chunks in flight (within-row throttle)


def build_nc(R=R, T=T, L=L, G=G, n_gaps=N_GAPS):
    import concourse.bacc as bacc
    import concourse.bass as bass
    import concourse.mybir as mybir
    from contextlib import ExitStack

    mult = mybir.AluOpType.mult
    add = mybir.AluOpType.add
    amin = mybir.AluOpType.min
    sub = mybir.AluOpType.subtract
    amod = mybir.AluOpType.mod

    P = 32  # partitions per row tile
    W = G // P  # 500 samples per partition (1000B in f16)
    V = L // P  # 750 samples per partition
    CF = min(1000, G // 4)
    CH = T // N_CHUNK
    assert P * W == G and P * V == L and 2 * V == 3 * W
    assert R == 4 and R * P == 128 and CH * N_CHUNK == T

    f16 = mybir.dt.float16
    f32 = mybir.dt.float32

    nc = bacc.Bacc()
    orig = nc.declare_dram_parameter("orig", [R, T], f16, isOutput=False)
    gen = nc.declare_dram_parameter("gen", [R, L], f16, isOutput=False)
    gaps = nc.declare_dram_parameter("gaps", [R, n_gaps], mybir.dt.int32, isOutput=False)
    out = nc.declare_dram_parameter("out", [R, T], f16, isOutput=True)
    # per-gap order-free scatter contents for rows 2,3 (+pad half)
    cbuf = {r: nc.dram_tensor(f"cbuf{r}", [n_gaps, 2 * G], f16) for r in (2, 3)}
    # DRAM staging of rows 2,3 faded segments (single-descriptor source)
    seg_d = {r: nc.dram_tensor(f"seg_d{r}", [G], f16) for r in (2, 3)}

    with ExitStack() as ctx:
        ec = ctx.enter_context
        g_sb = ec(nc.sbuf_tensor("g_sb", [128, V], f16))
        o_sb = ec(nc.sbuf_tensor("o_sb", [128, W], f16))
        bq = ec(nc.sbuf_tensor("bq", [128, W // 2], f16))
        it = ec(nc.sbuf_tensor("it", [128, W], mybir.dt.int32))
        pc = ec(nc.sbuf_tensor("pc", [128, 1], mybir.dt.int32))
        pcf = ec(nc.sbuf_tensor("pcf", [128, 1], f32))
        ft = ec(nc.sbuf_tensor("ft", [128, W], f32))
        w1 = ec(nc.sbuf_tensor("w1", [128, W], f32))
        fm = ec(nc.sbuf_tensor("fm", [128, W], f32))
        fm16 = ec(nc.sbuf_tensor("fm16", [128, W], f16))
        gaps_sb = ec(nc.sbuf_tensor("gaps_sb", [1, R * n_gaps], mybir.dt.int32))
        gf = ec(nc.sbuf_tensor("gf", [1, R * n_gaps], f32))
        d32 = ec(nc.sbuf_tensor("d32", [1, 15], f32))
        d_sb = ec(nc.sbuf_tensor("d_sb", [1, 15], mybir.dt.int32))

        ld_gaps = ec(nc.semaphore("ld_gaps"))
        ld_gen = ec(nc.semaphore("ld_gen"))
        io_sem = ec(nc.semaphore("io_sem"))
        vv = ec(nc.semaphore("vv"))
        cs = [ec(nc.semaphore(f"cs{r}")) for r in range(R)]
        ss = {r: ec(nc.semaphore(f"ss{r}")) for r in (0, 1)}
        sg = {r: ec(nc.semaphore(f"sg{r}")) for r in (2, 3)}
        ini = {r: ec(nc.semaphore(f"ini{r}")) for r in (2, 3)}
        rec = {r: ec(nc.semaphore(f"rec{r}")) for r in (2, 3)}
        fin = {r: ec(nc.semaphore(f"fin{r}")) for r in (2, 3)}
        block = ec(nc.Block())

        N_FM_OPS = 9  # vector ops producing fm16
        N_SEG_OPS = N_FM_OPS + 4  # ...through the final faded segment
        N_ALL_OPS = N_SEG_OPS + 4  # ...through the d-values for rows 2,3

        # d_sb layout: j in 0..6 -> row2 gap j ; j in 8..14 -> row3 gap j-8
        def d_idx(r, g):
            return {2: 0, 3: 8}[r] + g

        def chain_write(eng, r, g):
            """One ordered gap write for rows 0/1 (chain via ss[r])."""
            if True:
                with eng.register(f"off_{g}_{r}") as reg:
                    idx = r * n_gaps + g
                    eng.reg_load(reg, gaps_sb[0:1, idx : idx + 1])
                    off = eng.snap(reg, donate=True)
                    if g == 0:
                        eng.wait_ge(cs[r], 16 * N_CHUNK)  # row copy done
                    else:
                        eng.wait_ge(ss[r], 16 * g)  # prev gap write done
                    src = o_sb[r * P : (r + 1) * P, :]
                    eng.dma_start(
                        out=out[r][bass.ds(off, G)], in_=src,
                        max_dma_last_dim=2000,
                    ).then_inc(ss[r], 16)

        def interleaved_chains(eng):
            """Ordered chains for rows 0,1 interleaved so both rows' writes
            are concurrently in flight (prime row 0 by 3 links so the cs[1]
            wait can't head-of-line block row 0's early links)."""
            eng.wait_ge(vv, N_SEG_OPS)
            eng.wait_ge(ld_gaps, 16)
            order = [(0, 0), (0, 1), (0, 2)]
            for g in range(3, n_gaps):
                order += [(0, g), (1, g - 3)]
            order += [(1, g) for g in range(n_gaps - 3, n_gaps)]
            assert len(order) == 2 * n_gaps
            for r, g in order:
                chain_write(eng, r, g)

        def _dead_chain(eng, r):
            for g in range(n_gaps):
                with eng.register(f"off_{g}_{r}") as reg:
                    idx = r * n_gaps + g
                    eng.reg_load(reg, gaps_sb[0:1, idx : idx + 1])
                    off = eng.snap(reg, donate=True)
                    if g == 0:
                        eng.wait_ge(cs[r], 16 * N_CHUNK)  # row copy done
                    else:
                        eng.wait_ge(ss[r], 16 * g)  # prev gap write done
                    src = o_sb[r * P : (r + 1) * P, :]
                    eng.dma_start(
                        out=out[r][bass.ds(off, G)], in_=src,
                        max_dma_last_dim=2000,
                    ).then_inc(ss[r], 16)

        def finals_part(eng, r, gs):
            """Order-free writes for gap subset gs of row r: hoisted
            offset-register loads, one combined wait, back-to-back DMAs."""
            with ExitStack() as regs:
                offs = {}
                for g in gs:
                    reg = regs.enter_context(eng.register(f"fin_{r}_{g}"))
                    idx = r * n_gaps + g
                    eng.reg_load(reg, gaps_sb[0:1, idx : idx + 1])
                    offs[g] = eng.snap(reg, donate=True)
                eng.wait_ge(cs[r], 16 * N_CHUNK)  # row copy done
                eng.wait_ge(rec[r], 16 * (n_gaps - 1))  # all c_g final
                for g in gs:
                    eng.dma_start(
                        out=out[r][bass.ds(offs[g], G)], in_=cbuf[r][g][0:G],
                        max_dma_last_dim=2000,
                    ).then_inc(fin[r], 16)

        @block.sync
        def _(sync):
            # the bulk copy: out[r] <- orig[r], 3.84 MB/row, DRAM->DRAM,
            # 8 chunks per row, <= ~4 chunks in flight
            for r in range(R):
                for c in range(N_CHUNK):
                    if c >= MAX_INFLIGHT:
                        sync.wait_ge(cs[r], 16 * (c - MAX_INFLIGHT + 1))
                    sync.dma_start(
                        out=out[r][c * CH : (c + 1) * CH],
                        in_=orig[r][c * CH : (c + 1) * CH],
                    ).then_inc(cs[r], 16)
            finals_part(sync, 2, [3, 2, 1, 0])
            finals_part(sync, 3, [7, 6, 5, 4])

        @block.scalar
        def _(scalar):
            scalar.dma_start(
                out=g_sb[:], in_=gen[:].rearrange("r (p k) -> (r p) k", p=P)
            ).then_inc(ld_gen, 16)
            scalar.dma_start(
                out=gaps_sb[:], in_=gaps[:].rearrange("r g -> (r g)")[None, :]
            ).then_inc(ld_gaps, 16)
            interleaved_chains(scalar)
            finals_part(scalar, 3, [3, 2, 1, 0])

        @block.gpsimd
        def _(gpsimd):
            gpsimd.iota(
                it[:], pattern=[[1, W]], base=0, channel_multiplier=0
            ).then_inc(io_sem, 1)  # it[p, j] = j
            gpsimd.iota(
                pc[:], pattern=[[1, 1]], base=0, channel_multiplier=1
            ).then_inc(io_sem, 1)  # pc[p, 0] = p
            # order-free scatter prep for rows 2,3: stage each segment to
            # DRAM once, then ONE broadcast-AP DMA per row initializes all
            # 8 c_g slots, then the backward recursion (rows interleaved).
            gpsimd.wait_ge(vv, N_SEG_OPS)
            for r in (3, 2):
                gpsimd.dma_start(
                    out=seg_d[r][0:G], in_=o_sb[r * P : (r + 1) * P, :]
                ).then_inc(sg[r], 16)
            for r in (3, 2):
                gpsimd.wait_ge(sg[r], 16)
                gpsimd.dma_start(
                    out=cbuf[r][:, 0:G],
                    in_=seg_d[r][0:G][None, :].to_broadcast([n_gaps, G]),
                    max_dma_last_dim=2000,
                ).then_inc(ini[r], 16)
            gpsimd.wait_ge(vv, N_ALL_OPS)
            for g in range(n_gaps - 2, -1, -1):
                for r in (3, 2):
                    with gpsimd.register(f"d_{r}_{g}") as reg:
                        j = d_idx(r, g)
                        gpsimd.reg_load(reg, d_sb[0:1, j : j + 1])
                        doff = gpsimd.snap(reg, donate=True)
                        if g == n_gaps - 2:
                            gpsimd.wait_ge(ini[r], 16)
                        else:
                            gpsimd.wait_ge(rec[r], 16 * (n_gaps - 2 - g))
                        gpsimd.dma_start(
                            out=cbuf[r][g][bass.ds(doff, G)],
                            in_=cbuf[r][g + 1][0:G],
                            max_dma_last_dim=2000,
                        ).then_inc(rec[r], 16)
            finals_part(gpsimd, 2, [7, 6, 5, 4])

        @block.vector
        def _(vector):
            nv = 0

            def chainv(inst):
                nonlocal nv
                nv += 1
                inst.then_inc(vv, 1)

            def vwait():
                vector.wait_ge(vv, nv)

            # fade multiplier tile on ALL 128 partitions directly:
            #   q = (p & 31)*W + j == within-row position (each row owns 32
            #   partitions, 32*W == G) -- no cross-partition replication.
            #   fm = min(min(q, G-1-q) / (CF-1), 1.0)
            # equals the reference linspace crossfade up to 1 ulp.
            vector.wait_ge(io_sem, 2)
            chainv(
                vector.tensor_scalar(
                    pc[:], pc[:], 31, None, mybir.AluOpType.bitwise_and
                )
            )
            vwait()
            chainv(vector.tensor_copy(pcf[:], pc[:]))  # int32 -> f32 cast
            chainv(vector.tensor_copy(ft[:], it[:]))  # int32 -> f32 cast
            vwait()
            chainv(vector.tensor_scalar_mul(pcf[:], pcf[:], float(W)))
            vwait()
            chainv(vector.tensor_scalar(ft[:], ft[:], pcf[:], None, add))
            vwait()
            chainv(vector.tensor_scalar(w1[:], ft[:], -1.0, float(G - 1), mult, add))
            vwait()
            chainv(vector.scalar_tensor_tensor(fm[:], ft[:], 1.0, w1[:], mult, amin))
            vwait()
            chainv(vector.tensor_scalar(fm[:], fm[:], 1.0 / (CF - 1), 1.0, mult, amin))
            vwait()
            chainv(vector.tensor_copy(fm16[:], fm[:]))  # f32 -> f16 cast
            assert nv == N_FM_OPS, (nv, N_FM_OPS)

            # interpolation stencil + fade, all 4 batch rows in one 128-part
            # tile (each row owns 32 partitions; the stride-3/stride-2
            # stencil never crosses a partition boundary since 750 = 3*250
            # and 500 = 2*250).
            vector.wait_ge(ld_gen, 16)
            g3 = g_sb[:].rearrange("p (k c) -> p k c", c=3)
            o2 = o_sb[:].rearrange("p (m c) -> p m c", c=2)
            a = g3[:, :, 0]
            b = g3[:, :, 1]
            cc = g3[:, :, 2]
            vwait()
            chainv(vector.tensor_scalar_mul(bq[:], b, 0.25))
            vwait()
            chainv(vector.scalar_tensor_tensor(o2[:, :, 0], a, 0.75, bq[:], mult, add))
            chainv(vector.scalar_tensor_tensor(o2[:, :, 1], cc, 0.75, bq[:], mult, add))
            vwait()
            chainv(
                vector.scalar_tensor_tensor(o_sb[:], o_sb[:], 1.0, fm16[:], mult, mult)
            )
            assert nv == N_SEG_OPS, (nv, N_SEG_OPS)

            # d-values for rows 2,3: d[g] = min(gap[g+1]-gap[g], G), exact
            # in f32 (gap starts < 2^24). d32[j] = gf[16+j+1] - gf[16+j],
            # j=0..14; j=0..6 are row2 gaps, j=8..14 row3 (j=7 crosses rows,
            # unused).
            vector.wait_ge(ld_gaps, 16)
            chainv(vector.tensor_copy(gf[:], gaps_sb[:]))  # int32 -> f32
            vwait()
            chainv(
                vector.scalar_tensor_tensor(
                    d32[:], gf[0:1, 17:32], 1.0, gf[0:1, 16:31], mult, sub
                )
            )
            vwait()
            chainv(vector.tensor_scalar(d32[:], d32[:], 1.0, float(G), mult, amin))
            vwait()
            chainv(vector.tensor_copy(d_sb[:], d32[:]))  # f32 -> int32
            assert nv == N_ALL_OPS, (nv, N_ALL_OPS)

    return nc


_NC_CACHE = {}


def _get_nc():
    if "nc" not in _NC_CACHE:
        nc = build_nc()
        nc.finalize()  # Bacc: register allocation + codegen passes
        _NC_CACHE["nc"] = nc
    return _NC_CACHE["nc"]


def make_in_maps(original_audio, generated_audio, gap_starts):
    """Shard + dtype-convert full inputs into per-core in_maps."""
    original_audio = np.asarray(original_audio, dtype=np.float32)
    generated_audio = np.asarray(generated_audio, dtype=np.float32)
    gap_starts = np.asarray(gap_starts, dtype=np.int32)
    assert original_audio.shape == (B, T)
    assert generated_audio.shape == (B, L)
    assert gap_starts.shape == (B, N_GAPS)

    orig16 = original_audio.astype(np.float16)
    gen16 = generated_audio.astype(np.float16)

    in_maps = []
    for c in range(N_CORES):
        sl = slice(c * R, (c + 1) * R)
        in_maps.append(
            {
                "orig": np.ascontiguousarray(orig16[sl]),
                "gen": np.ascontiguousarray(gen16[sl]),
                "gaps": np.ascontiguousarray(gap_starts[sl]),
            }
        )
    return in_maps


def gather_out(res):
    """Concatenate per-core f16 outputs into the full f32 output."""
    return np.concatenate(
        [res.results[c]["out"] for c in range(N_CORES)], axis=0
    ).astype(np.float32)


def kernel(original_audio, generated_audio, gap_starts, gap_length):
    from concourse.bass_utils import run_bass_kernel_spmd

    assert int(gap_length) == G
    nc = _get_nc()
    in_maps = make_in_maps(original_audio, generated_audio, gap_starts)
    res = run_bass_kernel_spmd(nc, in_maps, core_ids=list(range(N_CORES)))
    return gather_out(res)


# revision 30
# speedup vs baseline: 1.4428x; 1.4428x over previous
"""Trainium2 Bass kernel for nn_AudioSegmentHandler (scatter_memory).

Semantics (matches the reference):
  1. Linear-interpolate each row's generated_audio [24000] down to
     gap_length=16000 (torch F.interpolate align_corners=False). Since
     24000/16000 == 1.5 exactly, the gather pattern is a fixed stride-3
     / stride-2 stencil:
        out[2k]   = 0.75*g[3k]   + 0.25*g[3k+1]
        out[2k+1] = 0.25*g[3k+1] + 0.75*g[3k+2]
  2. Crossfade: first 1000 samples *= linspace(0,1,1000), last 1000
     *= linspace(1,0,1000).
  3. For each row, sequentially scatter-write the 16000-sample segment
     into the audio at the 8 (sorted) gap_starts offsets; later gaps
     overwrite earlier ones on overlap.

Distribution: pure data-parallel, batch 32 -> 8 NeuronCores x 4 rows.

Perf strategy (vs the f32 baseline at ~145us):
  - fp16 transport end-to-end (gate is rel_err < 2e-2; fp16 adds
    ~1e-3): halves the dominant DRAM->DRAM copy to 15.4MB per core.
  - Copy issued first (sync queue), chunked and throttled to <=4
    chunks in flight so other queues' small DMAs interleave within a
    couple of 60KB descriptors instead of a deep descriptor backlog.
  - Scatter rows 0,1 (whose copies finish early): ordered chains
    (overlap semantics), interleaved on the scalar queue; latency
    hides under the copy of later rows.
  - Scatter rows 2,3 (whose copies finish last): ORDER-FREE. In DRAM
    scratch we precompute per-gap contents c_g such that overlapping
    gap writes carry identical bytes: backward over the sorted gaps,
        c_7 = seg;  c_g = seg overwritten on [d_g, d_g+G) by c_{g+1},
    d_g = min(gap[g+1]-gap[g], G) (dynamic DMA offset; writes beyond
    G land in a pad half). All 8 writes of the row then commute, so
    after the row's copy lands they issue back-to-back with no
    inter-write semaphores -- and are split across two queues -- so
    the post-copy tail is ~4us instead of ~8x3us.
  - Engine instruction issue is ~0.6-1us each, so the structure
    minimizes per-queue instruction counts: the segment is staged once
    per row to DRAM (single-descriptor 32KB sources), c_g inits are a
    single broadcast-AP DMA per row, and final writes hoist their
    offset-register loads and use one combined semaphore wait.
"""

import numpy as np

B = 32
T = 1920000
L = 24000  # generated_audio length
G = 16000  # gap length
N_GAPS = 8
N_CORES = 8
R = B // N_CORES  # rows per core
N_CHUNK = 8  # copy chunks per row
MAX_INFLIGHT = 4  # copy chunks in flight (within-row throttle)


def build_nc(R=R, T=T, L=L, G=G, n_gaps=N_GAPS):
    import concourse.bacc as bacc
    import concourse.bass as bass
    import concourse.mybir as mybir
    from contextlib import ExitStack

    mult = mybir.AluOpType.mult
    add = mybir.AluOpType.add
    amin = mybir.AluOpType.min
    sub = mybir.AluOpType.subtract

    P = 32  # partitions per row tile
    W = G // P  # 500 samples per partition (1000B in f16)
    V = L // P  # 750 samples per partition
    CF = min(1000, G // 4)
    CH = T // N_CHUNK
    assert P * W == G and P * V == L and 2 * V == 3 * W
    assert R == 4 and R * P == 128 and CH * N_CHUNK == T

    f16 = mybir.dt.float16
    f32 = mybir.dt.float32

    nc = bacc.Bacc()
    orig = nc.declare_dram_parameter("orig", [R, T], f16, isOutput=False)
    gen = nc.declare_dram_parameter("gen", [R, L], f16, isOutput=False)
    gaps = nc.declare_dram_parameter("gaps", [R, n_gaps], mybir.dt.int32, isOutput=False)
    out = nc.declare_dram_parameter("out", [R, T], f16, isOutput=True)
    # per-gap order-free scatter contents for rows 2,3 (+pad half)
    cbuf = {r: nc.dram_tensor(f"cbuf{r}", [n_gaps, 2 * G], f16) for r in (2, 3)}
    # DRAM staging of rows 2,3 faded segments (single-descriptor source)
    seg_d = {r: nc.dram_tensor(f"seg_d{r}", [G], f16) for r in (2, 3)}

    with ExitStack() as ctx:
        ec = ctx.enter_context
        g_sb = ec(nc.sbuf_tensor("g_sb", [128, V], f16))
        o_sb = ec(nc.sbuf_tensor("o_sb", [128, W], f16))
        bq = ec(nc.sbuf_tensor("bq", [128, W // 2], f16))
        it = ec(nc.sbuf_tensor("it", [P, W], mybir.dt.int32))
        ft = ec(nc.sbuf_tensor("ft", [P, W], f32))
        w1 = ec(nc.sbuf_tensor("w1", [P, W], f32))
        fm = ec(nc.sbuf_tensor("fm", [P, W], f32))
        fm16 = ec(nc.sbuf_tensor("fm16", [P, W], f16))
        fmr = ec(nc.sbuf_tensor("fmr", [128, W], f16))
        gaps_sb = ec(nc.sbuf_tensor("gaps_sb", [1, R * n_gaps], mybir.dt.int32))
        gf = ec(nc.sbuf_tensor("gf", [1, R * n_gaps], f32))
        d32 = ec(nc.sbuf_tensor("d32", [1, 15], f32))
        d_sb = ec(nc.sbuf_tensor("d_sb", [1, 15], mybir.dt.int32))

        ld_gaps = ec(nc.semaphore("ld_gaps"))
        ld_gen = ec(nc.semaphore("ld_gen"))
        ld_fm = ec(nc.semaphore("ld_fm"))
        io_sem = ec(nc.semaphore("io_sem"))
        vv = ec(nc.semaphore("vv"))
        cs = [ec(nc.semaphore(f"cs{r}")) for r in range(R)]
        ss = {r: ec(nc.semaphore(f"ss{r}")) for r in (0, 1)}
        sg = {r: ec(nc.semaphore(f"sg{r}")) for r in (2, 3)}
        ini = {r: ec(nc.semaphore(f"ini{r}")) for r in (2, 3)}
        rec = {r: ec(nc.semaphore(f"rec{r}")) for r in (2, 3)}
        fin = {r: ec(nc.semaphore(f"fin{r}")) for r in (2, 3)}
        block = ec(nc.Block())

        N_FM_OPS = 5  # vector ops producing fm16
        N_SEG_OPS = N_FM_OPS + 4  # ...through the final faded segment
        N_ALL_OPS = N_SEG_OPS + 4  # ...through the d-values for rows 2,3

        # d_sb layout: j in 0..6 -> row2 gap j ; j in 8..14 -> row3 gap j-8
        def d_idx(r, g):
            return {2: 0, 3: 8}[r] + g

        def chain_write(eng, r, g):
            """One ordered gap write for rows 0/1 (chain via ss[r])."""
            with eng.register(f"off_{g}_{r}") as reg:
                idx = r * n_gaps + g
                eng.reg_load(reg, gaps_sb[0:1, idx : idx + 1])
                off = eng.snap(reg, donate=True)
                if g == 0:
                    eng.wait_ge(cs[r], 16 * N_CHUNK)  # row copy done
                else:
                    eng.wait_ge(ss[r], 16 * g)  # prev gap write done
                src = o_sb[r * P : (r + 1) * P, :]
                eng.dma_start(out=out[r][bass.ds(off, G)], in_=src).then_inc(
                    ss[r], 16
                )

        def finals_part(eng, regs, r, gs):
            """Preload offset regs for gap subset gs of row r (hoisted,
            before any blocking wait); returns a closure that later emits
            the waits + back-to-back order-free writes."""
            offs = {}
            for g in gs:
                reg = regs.enter_context(eng.register(f"fin_{r}_{g}"))
                idx = r * n_gaps + g
                eng.reg_load(reg, gaps_sb[0:1, idx : idx + 1])
                offs[g] = eng.snap(reg, donate=True)

            def emit():
                eng.wait_ge(cs[r], 16 * N_CHUNK)  # row copy done
                eng.wait_ge(rec[r], 16 * (n_gaps - 1))  # all c_g final
                for g in gs:
                    eng.dma_start(
                        out=out[r][bass.ds(offs[g], G)], in_=cbuf[r][g][0:G]
                    ).then_inc(fin[r], 16)

            return emit

        @block.sync
        def _(sync):
            # the bulk copy: out[r] <- orig[r], 3.84 MB/row, DRAM->DRAM,
            # 8 chunks per row, <= ~4 chunks in flight
            for r in range(R):
                for c in range(N_CHUNK):
                    if c >= MAX_INFLIGHT:
                        sync.wait_ge(cs[r], 16 * (c - MAX_INFLIGHT + 1))
                    sync.dma_start(
                        out=out[r][c * CH : (c + 1) * CH],
                        in_=orig[r][c * CH : (c + 1) * CH],
                    ).then_inc(cs[r], 16)
            sync.wait_ge(ld_gaps, 16)
            with ExitStack() as regs:
                emit2 = finals_part(sync, regs, 2, [3, 2, 1, 0])
                emit3 = finals_part(sync, regs, 3, [7, 6, 5, 4])
                emit2()
                emit3()

        @block.scalar
        def _(scalar):
            scalar.dma_start(
                out=gaps_sb[:], in_=gaps[:].rearrange("r g -> (r g)")[None, :]
            ).then_inc(ld_gaps, 16)
            scalar.dma_start(
                out=g_sb[:], in_=gen[:].rearrange("r (p k) -> (r p) k", p=P)
            ).then_inc(ld_gen, 16)
            # ordered chains for rows 0,1, interleaved so both rows' writes
            # are concurrently in flight (prime row 0 by 3 links so the
            # cs[1] wait can't head-of-line block row 0's early links)
            scalar.wait_ge(vv, N_SEG_OPS)
            with ExitStack() as regs:
                emit3 = finals_part(scalar, regs, 3, [3, 2, 1, 0])
                order = [(0, 0), (0, 1), (0, 2)]
                for g in range(3, n_gaps):
                    order += [(0, g), (1, g - 3)]
                order += [(1, g) for g in range(n_gaps - 3, n_gaps)]
                assert len(order) == 2 * n_gaps
                for r, g in order:
                    chain_write(scalar, r, g)
                emit3()

        @block.gpsimd
        def _(gpsimd):
            gpsimd.iota(
                it[:], pattern=[[1, W]], base=0, channel_multiplier=W
            ).then_inc(io_sem, 1)  # it[p, j] = p*W + j
            # replicate the [32,W] fade tile into all 4 row slots of fmr
            gpsimd.wait_ge(vv, N_FM_OPS)
            for r in range(R):
                gpsimd.dma_start(
                    out=fmr[r * P : (r + 1) * P, :], in_=fm16[:]
                ).then_inc(ld_fm, 16)
            # order-free scatter prep for rows 2,3: stage each segment to
            # DRAM once, then ONE broadcast-AP DMA per row initializes all
            # 8 c_g slots, then the backward recursion (rows interleaved).
            gpsimd.wait_ge(vv, N_SEG_OPS)
            for r in (3, 2):
                gpsimd.dma_start(
                    out=seg_d[r][0:G], in_=o_sb[r * P : (r + 1) * P, :]
                ).then_inc(sg[r], 16)
            for r in (3, 2):
                gpsimd.wait_ge(sg[r], 16)
                gpsimd.dma_start(
                    out=cbuf[r][:, 0:G],
                    in_=seg_d[r][0:G][None, :].to_broadcast([n_gaps, G]),
                ).then_inc(ini[r], 16)
            with ExitStack() as regs:
                emit2 = finals_part(gpsimd, regs, 2, [7, 6, 5, 4])
                gpsimd.wait_ge(vv, N_ALL_OPS)
                for g in range(n_gaps - 2, -1, -1):
                    for r in (3, 2):
                        with gpsimd.register(f"d_{r}_{g}") as reg:
                            j = d_idx(r, g)
                            gpsimd.reg_load(reg, d_sb[0:1, j : j + 1])
                            doff = gpsimd.snap(reg, donate=True)
                            if g == n_gaps - 2:
                                gpsimd.wait_ge(ini[r], 16)
                            else:
                                gpsimd.wait_ge(rec[r], 16 * (n_gaps - 2 - g))
                            gpsimd.dma_start(
                                out=cbuf[r][g][bass.ds(doff, G)],
                                in_=cbuf[r][g + 1][0:G],
                            ).then_inc(rec[r], 16)
                emit2()

        @block.vector
        def _(vector):
            nv = 0

            def chain(inst):
                nonlocal nv
                nv += 1
                inst.then_inc(vv, 1)

            def vwait():
                vector.wait_ge(vv, nv)

            # fade multiplier tile fm[p, j] for one 32-partition row:
            #   q = p*W + j (position in the 16000-long segment)
            #   fm = min(min(q, G-1-q) / (CF-1), 1.0)
            # equals the reference linspace crossfade up to 1 ulp.
            vector.wait_ge(io_sem, 1)
            chain(vector.tensor_copy(ft[:], it[:]))  # int32 -> f32 cast
            vwait()
            chain(vector.tensor_scalar(w1[:], ft[:], -1.0, float(G - 1), mult, add))
            vwait()
            chain(vector.scalar_tensor_tensor(fm[:], ft[:], 1.0, w1[:], mult, amin))
            vwait()
            chain(vector.tensor_scalar(fm[:], fm[:], 1.0 / (CF - 1), 1.0, mult, amin))
            vwait()
            chain(vector.tensor_copy(fm16[:], fm[:]))  # f32 -> f16 cast
            assert nv == N_FM_OPS, (nv, N_FM_OPS)

            # interpolation stencil + fade, all 4 batch rows in one 128-part
            # tile (each row owns 32 partitions; the stride-3/stride-2
            # stencil never crosses a partition boundary since 750 = 3*250
            # and 500 = 2*250).
            vector.wait_ge(ld_gen, 16)
            g3 = g_sb[:].rearrange("p (k c) -> p k c", c=3)
            o2 = o_sb[:].rearrange("p (m c) -> p m c", c=2)
            a = g3[:, :, 0]
            b = g3[:, :, 1]
            cc = g3[:, :, 2]
            vwait()
            chain(vector.tensor_scalar_mul(bq[:], b, 0.25))
            vwait()
            chain(vector.scalar_tensor_tensor(o2[:, :, 0], a, 0.75, bq[:], mult, add))
            chain(vector.scalar_tensor_tensor(o2[:, :, 1], cc, 0.75, bq[:], mult, add))
            vwait()
            vector.wait_ge(ld_fm, 16 * R)
            chain(
                vector.scalar_tensor_tensor(o_sb[:], o_sb[:], 1.0, fmr[:], mult, mult)
            )
            assert nv == N_SEG_OPS, (nv, N_SEG_OPS)

            # d-values for rows 2,3: d[g] = min(gap[g+1]-gap[g], G), exact
            # in f32 (gap starts < 2^24). d32[j] = gf[16+j+1] - gf[16+j],
            # j=0..14; j=0..6 are row2 gaps, j=8..14 row3 (j=7 crosses rows,
            # unused).
            vector.wait_ge(ld_gaps, 16)
            chain(vector.tensor_copy(gf[:], gaps_sb[:]))  # int32 -> f32
            vwait()
            chain(
                vector.scalar_tensor_tensor(
                    d32[:], gf[0:1, 17:32], 1.0, gf[0:1, 16:31], mult, sub
                )
            )
            vwait()
            chain(vector.tensor_scalar(d32[:], d32[:], 1.0, float(G), mult, amin))
            vwait()
            chain(vector.tensor_copy(d_sb[:], d32[:]))  # f32 -> int32
            assert nv == N_ALL_OPS, (nv, N_ALL_OPS)

    return nc


_NC_CACHE = {}


def _get_nc():
    if "nc" not in _NC_CACHE:
        nc = build_nc()
        nc.finalize()  # Bacc: register allocation + codegen passes
        _NC_CACHE["nc"] = nc
    return _NC_CACHE["nc"]


def make_in_maps(original_audio, generated_audio, gap_starts):
    """Shard + dtype-convert full inputs into per-core in_maps."""
    original_audio = np.asarray(original_audio, dtype=np.float32)
    generated_audio = np.asarray(generated_audio, dtype=np.float32)
    gap_starts = np.asarray(gap_starts, dtype=np.int32)
    assert original_audio.shape == (B, T)
    assert generated_audio.shape == (B, L)
    assert gap_starts.shape == (B, N_GAPS)

    orig16 = original_audio.astype(np.float16)
    gen16 = generated_audio.astype(np.float16)

    in_maps = []
    for c in range(N_CORES):
        sl = slice(c * R, (c + 1) * R)
        in_maps.append(
            {
                "orig": np.ascontiguousarray(orig16[sl]),
                "gen": np.ascontiguousarray(gen16[sl]),
                "gaps": np.ascontiguousarray(gap_starts[sl]),
            }
        )
    return in_maps


def gather_out(res):
    """Concatenate per-core f16 outputs into the full f32 output."""
    return np.concatenate(
        [res.results[c]["out"] for c in range(N_CORES)], axis=0
    ).astype(np.float32)


def kernel(original_audio, generated_audio, gap_starts, gap_length):
    from concourse.bass_utils import run_bass_kernel_spmd

    assert int(gap_length) == G
    nc = _get_nc()
    in_maps = make_in_maps(original_audio, generated_audio, gap_starts)
    res = run_bass_kernel_spmd(nc, in_maps, core_ids=list(range(N_CORES)))
    return gather_out(res)
